# revision 3
# baseline (speedup 1.0000x reference)
"""Center-loss kernel for Trainium2 (8 NeuronCores, SPMD data-parallel).

Math: with per-class sums S_c = sum_{i: l_i=c} x_i, counts N_c, and
M_c = max(N_c, 1), the reference loss

    mean((centroid[l] - x)^2)  with centroid_c = S_c / M_c

expands to

    ( sum(x^2) - sum_c ||S_c||^2 / M_c ) / (n*d)

(the N_c = 0 case contributes 0 to both forms). So one pass over the
features suffices: per-class sums + counts + global sum of squares.

Device work per core (shard of 32768 rows x 256 cols, f32):
  - DMA 2 MiB tiles [128, 16*256]  (partition = sample dim)
  - ACT: Square with accum_out -> per-partition running sum of x^2
  - DVE: one-hot [128, 64] per 128-sample group via is_equal(iota, label)
  - PE : psum_sums[64,256] += onehot^T @ x_group   (PSUM accumulation)
         psum_cnt [64,1]   += onehot^T @ ones
Host: reduce the tiny per-core outputs and finish the scalar in f64.
"""

import numpy as np
from contextlib import ExitStack

import concourse.bass as bass
import concourse.bacc as bacc
import concourse.mybir as mybir
import concourse.tile as tile
from concourse.bass_utils import run_bass_kernel_spmd

# Hardcoded problem shape (contract: kernel.py is self-contained).
N, D = 262144, 256
N_CLASS = 64
N_CORES = 8
NS = N // N_CORES            # 32768 rows per core
P = 128                      # SBUF partitions = contraction dim per group
GROUPS = NS // P             # 256 groups of 128 rows per core
G_PER_TILE = 16              # one DMA tile = [128, 16*256] f32 = 2 MiB
N_TILES = GROUPS // G_PER_TILE

_built = None
last_results = None          # BassKernelResults of most recent run (for test.py)


def _build():
    nc = bacc.Bacc("TRN2", num_devices=N_CORES)
    x = nc.dram_tensor("x", [NS, D], mybir.dt.float32, kind="ExternalInput")
    lab = nc.dram_tensor("lab", [P, GROUPS], mybir.dt.float32, kind="ExternalInput")
    iota = nc.dram_tensor("iota", [P, N_CLASS], mybir.dt.float32, kind="ExternalInput")
    out_cls = nc.dram_tensor(
        "out_cls", [N_CLASS, D + 1], mybir.dt.float32, kind="ExternalOutput"
    )
    out_sq = nc.dram_tensor(
        "out_sq", [P, N_TILES], mybir.dt.float32, kind="ExternalOutput"
    )

    with ExitStack() as ctx:
        tc = ctx.enter_context(tile.TileContext(nc))
        singles = ctx.enter_context(tc.tile_pool(name="singles", bufs=1))
        xpool = ctx.enter_context(tc.tile_pool(name="xpool", bufs=3))
        ohpool = ctx.enter_context(tc.tile_pool(name="ohpool", bufs=4))
        psum = ctx.enter_context(tc.tile_pool(name="psum", bufs=1, space="PSUM"))

        lab_sb = singles.tile([P, GROUPS], mybir.dt.float32)
        nc.sync.dma_start(out=lab_sb[:], in_=lab.ap())
        iota_sb = singles.tile([P, N_CLASS], mybir.dt.float32)
        nc.sync.dma_start(out=iota_sb[:], in_=iota.ap())
        ones_sb = singles.tile([P, 1], mybir.dt.float32)
        nc.vector.memset(ones_sb[:], 1.0)
        sq_acc = singles.tile([P, N_TILES], mybir.dt.float32)
        sq_scr = singles.tile([P, G_PER_TILE, D], mybir.dt.float32)

        ps_sums = psum.tile([N_CLASS, D], mybir.dt.float32)
        ps_cnt = psum.tile([N_CLASS, 1], mybir.dt.float32)

        # x row (t*16+g)*128 + p  ->  xt[p, g, :] for tile t
        xr = x.ap().rearrange("(t g p) d -> t p g d", g=G_PER_TILE, p=P)
        for t in range(N_TILES):
            xt = xpool.tile([P, G_PER_TILE, D], mybir.dt.float32)
            nc.sync.dma_start(out=xt[:], in_=xr[t])
            nc.scalar.activation(
                out=sq_scr[:],
                in_=xt[:],
                func=mybir.ActivationFunctionType.Square,
                accum_out=sq_acc[:, t : t + 1],
            )
            for g in range(G_PER_TILE):
                gi = t * G_PER_TILE + g
                oh = ohpool.tile([P, N_CLASS], mybir.dt.float32)
                nc.vector.tensor_scalar(
                    out=oh[:],
                    in0=iota_sb[:],
                    scalar1=lab_sb[:, gi : gi + 1],
                    scalar2=None,
                    op0=mybir.AluOpType.is_equal,
                )
                nc.tensor.matmul(
                    out=ps_sums[:],
                    lhsT=oh[:],
                    rhs=xt[:, g, :],
                    start=(gi == 0),
                    stop=(gi == GROUPS - 1),
                )
                nc.tensor.matmul(
                    out=ps_cnt[:],
                    lhsT=oh[:],
                    rhs=ones_sb[:],
                    start=(gi == 0),
                    stop=(gi == GROUPS - 1),
                )

        out_sb = singles.tile([N_CLASS, D + 1], mybir.dt.float32)
        nc.vector.tensor_copy(out_sb[:, 0:D], ps_sums[:])
        nc.vector.tensor_copy(out_sb[:, D : D + 1], ps_cnt[:])
        nc.sync.dma_start(out=out_cls.ap(), in_=out_sb[:])
        nc.sync.dma_start(out=out_sq.ap(), in_=sq_acc[:])
    nc.compile()
    return nc


def kernel(s_feature, s_labels):
    global _built, last_results
    s_feature = np.ascontiguousarray(np.asarray(s_feature), dtype=np.float32)
    s_labels = np.asarray(s_labels)

    if _built is None:
        _built = _build()
    nc = _built

    iota_np = np.ascontiguousarray(
        np.broadcast_to(np.arange(N_CLASS, dtype=np.float32), (P, N_CLASS))
    )
    in_maps = []
    for c in range(N_CORES):
        xs = s_feature[c * NS : (c + 1) * NS]
        ls = s_labels[c * NS : (c + 1) * NS]
        lab_t = np.ascontiguousarray(
            np.asarray(ls).reshape(GROUPS, P).T.astype(np.float32)
        )
        in_maps.append({"x": xs, "lab": lab_t, "iota": iota_np})

    try:
        last_results = run_bass_kernel_spmd(nc, in_maps, core_ids=list(range(N_CORES)))
    except ModuleNotFoundError:
        # BASS_TRACE requested but the axon NTFF hook isn't present in this
        # container; rerun with tracing hard-disabled.
        import os

        os.environ["BASS_NEVER_TRACE"] = "1"
        last_results = run_bass_kernel_spmd(nc, in_maps, core_ids=list(range(N_CORES)))

    sums = np.zeros((N_CLASS, D), dtype=np.float64)
    counts = np.zeros((N_CLASS,), dtype=np.float64)
    s2 = 0.0
    for r in last_results.results:
        oc = np.asarray(r["out_cls"], dtype=np.float64)
        sums += oc[:, :D]
        counts += oc[:, D]
        s2 += float(np.asarray(r["out_sq"], dtype=np.float64).sum())

    denom = np.maximum(counts, 1.0)
    corr = float(np.sum(np.sum(sums * sums, axis=1) / denom))
    loss = (s2 - corr) / (float(N) * float(D))
    return np.array(loss, dtype=np.float32)


# revision 6
# speedup vs baseline: 4.6986x; 4.6986x over previous
"""Center-loss kernel for Trainium2 (8 NeuronCores, SPMD data-parallel).

Math: with per-class sums S_c = sum_{i: l_i=c} x_i, counts N_c, and
M_c = max(N_c, 1), the reference loss

    mean((centroid[l] - x)^2)  with centroid_c = S_c / M_c

expands to

    ( sum(x^2) - sum_c ||S_c||^2 / M_c ) / (n*d)

(the N_c = 0 case contributes 0 to both forms). So one pass over the
features suffices: per-class sums + counts + global sum of squares.

Device work per core (shard of 32768 rows x 256 cols, f32):
  - DMA 2 MiB tiles [128, 16*256]  (partition = sample dim)
  - ACT: Square with accum_out -> per-partition running sum of x^2
  - DVE: one-hot [128, 64] per 128-sample group via is_equal(iota, label)
  - PE : psum_sums[64,256] += onehot^T @ x_group   (PSUM accumulation)
         psum_cnt [64,1]   += onehot^T @ ones
Host: reduce the tiny per-core outputs and finish the scalar in f64.
"""

import numpy as np
from contextlib import ExitStack

import concourse.bass as bass
import concourse.bacc as bacc
import concourse.mybir as mybir
import concourse.tile as tile
from concourse.bass_utils import run_bass_kernel_spmd

# Hardcoded problem shape (contract: kernel.py is self-contained).
N, D = 262144, 256
N_CLASS = 64
N_CORES = 8
NS = N // N_CORES            # 32768 rows per core
P = 128                      # SBUF partitions = contraction dim per group
GROUPS = NS // P             # 256 groups of 128 rows per core
G_PER_TILE = 16              # one DMA tile = [128, 16*256] f32 = 2 MiB
N_TILES = GROUPS // G_PER_TILE

_built = None
last_results = None          # BassKernelResults of most recent run (for test.py)


def _build(repeats=1):
    nc = bacc.Bacc("TRN2", num_devices=N_CORES)
    x = nc.dram_tensor("x", [NS, D], mybir.dt.float32, kind="ExternalInput")
    lab = nc.dram_tensor("lab", [P, GROUPS], mybir.dt.float32, kind="ExternalInput")
    iota = nc.dram_tensor("iota", [P, N_CLASS], mybir.dt.float32, kind="ExternalInput")
    out_cls = nc.dram_tensor(
        "out_cls", [N_CLASS, D + 1], mybir.dt.float32, kind="ExternalOutput"
    )
    out_sq = nc.dram_tensor(
        "out_sq", [P, N_TILES], mybir.dt.float32, kind="ExternalOutput"
    )

    with ExitStack() as ctx:
        tc = ctx.enter_context(tile.TileContext(nc))
        singles = ctx.enter_context(tc.tile_pool(name="singles", bufs=1))
        xpool = ctx.enter_context(tc.tile_pool(name="xpool", bufs=3))
        ohpool = ctx.enter_context(tc.tile_pool(name="ohpool", bufs=4))
        psum = ctx.enter_context(tc.tile_pool(name="psum", bufs=1, space="PSUM"))

        lab_sb = singles.tile([P, GROUPS], mybir.dt.float32)
        nc.sync.dma_start(out=lab_sb[:], in_=lab.ap())
        iota_sb = singles.tile([P, N_CLASS], mybir.dt.float32)
        nc.sync.dma_start(out=iota_sb[:], in_=iota.ap())
        ones_sb = singles.tile([P, 1], mybir.dt.float32)
        nc.vector.memset(ones_sb[:], 1.0)
        sq_acc = singles.tile([P, N_TILES], mybir.dt.float32)
        sq_scr = singles.tile([P, G_PER_TILE, D], mybir.dt.float32)

        ps_sums = psum.tile([N_CLASS, D], mybir.dt.float32)
        ps_cnt = psum.tile([N_CLASS, 1], mybir.dt.float32)

        # Partition p holds the shard's rows [p*256, (p+1)*256) flattened, so
        # every tile DMA is 128 contiguous 16 KiB chunks. Group gi = t*16+g is
        # sample p*256 + gi of partition p; labels arrive as the matching
        # [128, 256] = labels.reshape(128, 256) with no host transpose.
        xr = x.ap().rearrange("(p r) d -> p r d", p=P)
        for rep in range(repeats):
            for t in range(N_TILES):
                xt = xpool.tile([P, G_PER_TILE, D], mybir.dt.float32)
                nc.sync.dma_start(
                    out=xt[:], in_=xr[:, t * G_PER_TILE : (t + 1) * G_PER_TILE, :]
                )
                nc.scalar.activation(
                    out=sq_scr[:],
                    in_=xt[:],
                    func=mybir.ActivationFunctionType.Square,
                    accum_out=sq_acc[:, t : t + 1],
                )
                for g in range(G_PER_TILE):
                    gi = t * G_PER_TILE + g
                    oh = ohpool.tile([P, N_CLASS], mybir.dt.float32)
                    nc.vector.tensor_scalar(
                        out=oh[:],
                        in0=iota_sb[:],
                        scalar1=lab_sb[:, gi : gi + 1],
                        scalar2=None,
                        op0=mybir.AluOpType.is_equal,
                    )
                    nc.tensor.matmul(
                        out=ps_sums[:],
                        lhsT=oh[:],
                        rhs=xt[:, g, :],
                        start=(gi == 0),
                        stop=(gi == GROUPS - 1),
                    )
                    nc.tensor.matmul(
                        out=ps_cnt[:],
                        lhsT=oh[:],
                        rhs=ones_sb[:],
                        start=(gi == 0),
                        stop=(gi == GROUPS - 1),
                    )

        out_sb = singles.tile([N_CLASS, D + 1], mybir.dt.float32)
        nc.vector.tensor_copy(out_sb[:, 0:D], ps_sums[:])
        nc.vector.tensor_copy(out_sb[:, D : D + 1], ps_cnt[:])
        nc.sync.dma_start(out=out_cls.ap(), in_=out_sb[:])
        nc.sync.dma_start(out=out_sq.ap(), in_=sq_acc[:])
    nc.compile()
    return nc


def kernel(s_feature, s_labels):
    global _built, last_results
    s_feature = np.ascontiguousarray(np.asarray(s_feature), dtype=np.float32)
    s_labels = np.asarray(s_labels)

    if _built is None:
        _built = _build()
    nc = _built

    iota_np = np.ascontiguousarray(
        np.broadcast_to(np.arange(N_CLASS, dtype=np.float32), (P, N_CLASS))
    )
    in_maps = []
    for c in range(N_CORES):
        xs = s_feature[c * NS : (c + 1) * NS]
        ls = s_labels[c * NS : (c + 1) * NS]
        lab_t = np.ascontiguousarray(np.asarray(ls).reshape(P, GROUPS).astype(np.float32))
        in_maps.append({"x": xs, "lab": lab_t, "iota": iota_np})

    try:
        last_results = run_bass_kernel_spmd(nc, in_maps, core_ids=list(range(N_CORES)))
    except ModuleNotFoundError:
        # BASS_TRACE requested but the axon NTFF hook isn't present in this
        # container; rerun with tracing hard-disabled.
        import os

        os.environ["BASS_NEVER_TRACE"] = "1"
        last_results = run_bass_kernel_spmd(nc, in_maps, core_ids=list(range(N_CORES)))

    sums = np.zeros((N_CLASS, D), dtype=np.float64)
    counts = np.zeros((N_CLASS,), dtype=np.float64)
    s2 = 0.0
    for r in last_results.results:
        oc = np.asarray(r["out_cls"], dtype=np.float64)
        sums += oc[:, :D]
        counts += oc[:, D]
        s2 += float(np.asarray(r["out_sq"], dtype=np.float64).sum())

    denom = np.maximum(counts, 1.0)
    corr = float(np.sum(np.sum(sums * sums, axis=1) / denom))
    loss = (s2 - corr) / (float(N) * float(D))
    return np.array(loss, dtype=np.float32)
